# revision 8
# baseline (speedup 1.0000x reference)
"""CayleyLinear Trainium2 kernel.

Computes out = x @ Q + bias where Q = (I-A)^-1 (I+A) is the Cayley
transform of the skew-symmetric matrix built from `angles`.

Strategy (data-parallel over the batch dim, 8 NeuronCores):
  - Host: build A, solve for Q (1024x1024 — tiny vs the 68.7 GFLOP matmul),
    replicate Q/bias to every core, shard x along batch (8 -> 1 per core).
  - Host: pre-transpose each x shard to [K=1024, S=4096] so the contraction
    dim lands on SBUF partitions (avoids 256 on-device PE transposes/core).
  - Device: out[s, j] = sum_k xT[k, s] * Q[k, j] via 128x128x512 matmuls in
    float32r (full PE rate at free-dim >= 256), accumulating 8 k-tiles in
    PSUM, fused bias-add on the PSUM->SBUF copy (DVE), then DMA out.
"""

import numpy as np

DIM = 1024
B = 8
S = 4096
N_CORES = 8
P = 128
KT = DIM // P  # 8 k-tiles
S_SLAB = 512  # tokens DMA'd per slab
N_SLABS = S // S_SLAB  # 8
M_PER_SLAB = S_SLAB // P  # 4 matmul row-blocks per slab

_compiled_nc = None


def _build_kernel():
    import concourse.bass as bass
    import concourse.mybir as mybir
    import concourse.tile as tile
    from concourse import bacc

    f32 = mybir.dt.float32
    f32r = mybir.dt.float32r

    nc = bacc.Bacc(
        "TRN2", target_bir_lowering=False, debug=False, num_devices=N_CORES
    )

    xT_d = nc.dram_tensor("xT", [DIM, S], f32, kind="ExternalInput").ap()
    q_d = nc.dram_tensor("q", [DIM, DIM], f32, kind="ExternalInput").ap()
    bias_d = nc.dram_tensor("biasr", [P, DIM], f32, kind="ExternalInput").ap()
    out_d = nc.dram_tensor("out", [S, DIM], f32, kind="ExternalOutput").ap()

    xT_r = xT_d.rearrange("(kt p) s -> p kt s", p=P)  # [128, 8, 4096]
    q_r = q_d.rearrange("(kt p) j -> p kt j", p=P)  # [128, 8, 1024]

    with tile.TileContext(nc) as tc:
        with (
            tc.tile_pool(name="qpool", bufs=1) as qpool,
            tc.tile_pool(name="bpool", bufs=1) as bpool,
            tc.tile_pool(name="xpool", bufs=4) as xpool,
            tc.tile_pool(name="opool", bufs=4) as opool,
            tc.tile_pool(name="psum", bufs=1, space="PSUM") as psumpool,
        ):
            bias_t = bpool.tile([P, DIM], f32)
            nc.gpsimd.dma_start(bias_t[:], bias_d[:])

            # Q tiles load per k-tile on the gpsimd queue, x tiles on the
            # sync queue (parallel issue). The matmul loop is kt-major so
            # the first matmuls only need the first-arriving tiles, and
            # Q's 4MB arrival overlaps a full slab of PE work.
            q_tiles = [None] * KT

            for slab in range(N_SLABS):
                xts = []
                for kt in range(KT):
                    if slab == 0:
                        qt = qpool.tile([P, DIM], f32r, tag=f"q{kt}")
                        nc.gpsimd.dma_start(qt[:], q_r[:, kt, :].bitcast(f32r))
                        q_tiles[kt] = qt
                    xt = xpool.tile([P, S_SLAB], f32r, tag=f"x{kt}")
                    nc.sync.dma_start(
                        xt[:],
                        xT_r[
                            :, kt, slab * S_SLAB : (slab + 1) * S_SLAB
                        ].bitcast(f32r),
                    )
                    xts.append(xt)
                # 4 m-groups accumulate concurrently in 8 PSUM banks
                pss = [
                    psumpool.tile([P, DIM], f32, tag=f"ps{m}", name=f"ps{m}")
                    for m in range(M_PER_SLAB)
                ]
                for kt in range(KT):
                    for m in range(M_PER_SLAB):
                        lhs = xts[kt][:, m * P : (m + 1) * P]
                        for jh in range(2):
                            nc.tensor.matmul(
                                pss[m][:, jh * 512 : (jh + 1) * 512],
                                lhs,
                                q_tiles[kt][:, jh * 512 : (jh + 1) * 512],
                                start=(kt == 0),
                                stop=(kt == KT - 1),
                            )
                for m in range(M_PER_SLAB):
                    ot = opool.tile([P, DIM], f32)
                    nc.vector.tensor_add(ot[:], pss[m][:], bias_t[:])
                    sblk = slab * M_PER_SLAB + m
                    nc.scalar.dma_start(
                        out_d[sblk * P : (sblk + 1) * P, :], ot[:]
                    )

    nc.compile()
    return nc


def _get_nc():
    global _compiled_nc
    if _compiled_nc is None:
        _compiled_nc = _build_kernel()
    return _compiled_nc


def _cayley_q(angles: np.ndarray) -> np.ndarray:
    A = np.zeros((DIM, DIM), dtype=np.float64)
    iu = np.triu_indices(DIM, k=1)
    A[iu] = angles.astype(np.float64)
    A = A - A.T
    I = np.eye(DIM, dtype=np.float64)
    return np.linalg.solve(I - A, I + A).astype(np.float32)


def _run(inputs: dict, trace: bool = False, tmpdir: str | None = None):
    from concourse.bass_utils import run_bass_kernel_spmd

    x = np.asarray(inputs["x"], dtype=np.float32)
    angles = np.asarray(inputs["angles"], dtype=np.float32)
    bias = np.asarray(inputs["bias"], dtype=np.float32)

    Q = _cayley_q(angles)
    bias_rep = np.ascontiguousarray(
        np.broadcast_to(bias.astype(np.float32), (P, DIM))
    )
    in_maps = []
    for b in range(B):
        xT = np.ascontiguousarray(x[b].T)  # [1024, 4096]
        in_maps.append({"xT": xT, "q": Q, "biasr": bias_rep})

    nc = _get_nc()
    res = run_bass_kernel_spmd(
        nc, in_maps, list(range(N_CORES)), trace=trace, tmpdir=tmpdir
    )
    out = np.stack([res.results[b]["out"] for b in range(B)], axis=0)
    return out, res


def kernel(x, angles, bias):
    out, _ = _run({"x": x, "angles": angles, "bias": bias})
    return out


# revision 9
# speedup vs baseline: 1.3091x; 1.3091x over previous
"""CayleyLinear Trainium2 kernel.

Computes out = x @ Q + bias where Q = (I-A)^-1 (I+A) is the Cayley
transform of the skew-symmetric matrix built from `angles`.

Strategy (data-parallel over the batch dim, 8 NeuronCores):
  - Host: build A, solve for Q (1024x1024 — tiny vs the 68.7 GFLOP matmul),
    replicate Q/bias to every core, shard x along batch (8 -> 1 per core).
  - Host: pre-transpose each x shard to [K=1024, S=4096] so the contraction
    dim lands on SBUF partitions (avoids 256 on-device PE transposes/core).
  - Device: out[s, j] = sum_k xT[k, s] * Q[k, j] via 128x128x512 matmuls in
    float32r (full PE rate at free-dim >= 256), accumulating 8 k-tiles in
    PSUM, fused bias-add on the PSUM->SBUF copy (DVE), then DMA out.
"""

import numpy as np

DIM = 1024
B = 8
S = 4096
N_CORES = 8
P = 128
KT = DIM // P  # 8 k-tiles
S_SLAB = 512  # tokens DMA'd per slab
N_SLABS = S // S_SLAB  # 8
M_PER_SLAB = S_SLAB // P  # 4 matmul row-blocks per slab

_compiled_nc = None


def _build_kernel():
    import concourse.bass as bass
    import concourse.mybir as mybir
    import concourse.tile as tile
    from concourse import bacc

    f32 = mybir.dt.float32
    f32r = mybir.dt.float32r

    nc = bacc.Bacc(
        "TRN2", target_bir_lowering=False, debug=False, num_devices=N_CORES
    )

    xT_d = nc.dram_tensor("xT", [DIM, S], f32, kind="ExternalInput").ap()
    q_d = nc.dram_tensor("q", [DIM, DIM], f32, kind="ExternalInput").ap()
    bias_d = nc.dram_tensor("biasr", [P, DIM], f32, kind="ExternalInput").ap()
    out_d = nc.dram_tensor("out", [S, DIM], f32, kind="ExternalOutput").ap()

    xT_r = xT_d.rearrange("(kt p) s -> p kt s", p=P)  # [128, 8, 4096]
    q_r = q_d.rearrange("(kt p) j -> p kt j", p=P)  # [128, 8, 1024]

    with tile.TileContext(nc) as tc:
        with (
            tc.tile_pool(name="qpool", bufs=1) as qpool,
            tc.tile_pool(name="bpool", bufs=1) as bpool,
            tc.tile_pool(name="xpool", bufs=4) as xpool,
            tc.tile_pool(name="opool", bufs=4) as opool,
            tc.tile_pool(name="psum", bufs=1, space="PSUM") as psumpool,
        ):
            # Q tiles load per k-tile on the gpsimd queue, x tiles on the
            # sync queue (parallel issue). Slab 0 runs kt-major so the
            # first matmuls only need the first-arriving tiles and Q's
            # 4MB arrival overlaps a full slab of PE work; later slabs
            # (x prefetched) run m-outer so DVE drains and output stores
            # spread evenly and the kernel tail stays short.
            q_tiles = [None] * KT

            def mm(ps, xts, kt, m):
                lhs = xts[kt][:, m * P : (m + 1) * P]
                for jh in range(2):
                    nc.tensor.matmul(
                        ps[:, jh * 512 : (jh + 1) * 512],
                        lhs,
                        q_tiles[kt][:, jh * 512 : (jh + 1) * 512],
                        start=(kt == 0),
                        stop=(kt == KT - 1),
                    )

            def drain(pss, m, slab):
                ot = opool.tile([P, DIM], f32, name="ot")
                nc.vector.tensor_add(ot[:], pss[m][:], bias_t[:])
                sblk = slab * M_PER_SLAB + m
                nc.scalar.dma_start(out_d[sblk * P : (sblk + 1) * P, :], ot[:])

            bias_t = bpool.tile([P, DIM], f32)

            for slab in range(N_SLABS):
                xts = []
                for kt in range(KT):
                    if slab == 0:
                        qt = qpool.tile([P, DIM], f32r, tag=f"q{kt}")
                        nc.gpsimd.dma_start(qt[:], q_r[:, kt, :].bitcast(f32r))
                        q_tiles[kt] = qt
                        if kt == 0:
                            nc.gpsimd.dma_start(bias_t[:], bias_d[:])
                    xt = xpool.tile([P, S_SLAB], f32r, tag=f"x{kt}")
                    nc.sync.dma_start(
                        xt[:],
                        xT_r[
                            :, kt, slab * S_SLAB : (slab + 1) * S_SLAB
                        ].bitcast(f32r),
                    )
                    xts.append(xt)
                pss = [
                    psumpool.tile([P, DIM], f32, tag=f"ps{m}", name=f"ps{m}")
                    for m in range(M_PER_SLAB)
                ]
                if slab == 0:
                    # kt-major: 4 m-groups accumulate in 8 PSUM banks
                    for kt in range(KT):
                        for m in range(M_PER_SLAB):
                            mm(pss[m], xts, kt, m)
                    for m in range(M_PER_SLAB):
                        drain(pss, m, slab)
                else:
                    for m in range(M_PER_SLAB):
                        for kt in range(KT):
                            mm(pss[m], xts, kt, m)
                        drain(pss, m, slab)

    nc.compile()
    return nc


def _get_nc():
    global _compiled_nc
    if _compiled_nc is None:
        _compiled_nc = _build_kernel()
    return _compiled_nc


def _cayley_q(angles: np.ndarray) -> np.ndarray:
    A = np.zeros((DIM, DIM), dtype=np.float64)
    iu = np.triu_indices(DIM, k=1)
    A[iu] = angles.astype(np.float64)
    A = A - A.T
    I = np.eye(DIM, dtype=np.float64)
    return np.linalg.solve(I - A, I + A).astype(np.float32)


def _run(inputs: dict, trace: bool = False, tmpdir: str | None = None):
    from concourse.bass_utils import run_bass_kernel_spmd

    x = np.asarray(inputs["x"], dtype=np.float32)
    angles = np.asarray(inputs["angles"], dtype=np.float32)
    bias = np.asarray(inputs["bias"], dtype=np.float32)

    Q = _cayley_q(angles)
    bias_rep = np.ascontiguousarray(
        np.broadcast_to(bias.astype(np.float32), (P, DIM))
    )
    in_maps = []
    for b in range(B):
        xT = np.ascontiguousarray(x[b].T)  # [1024, 4096]
        in_maps.append({"xT": xT, "q": Q, "biasr": bias_rep})

    nc = _get_nc()
    res = run_bass_kernel_spmd(
        nc, in_maps, list(range(N_CORES)), trace=trace, tmpdir=tmpdir
    )
    out = np.stack([res.results[b]["out"] for b in range(B)], axis=0)
    return out, res


def kernel(x, angles, bias):
    out, _ = _run({"x": x, "angles": angles, "bias": bias})
    return out
